# revision 2
# baseline (speedup 1.0000x reference)
"""BiLinearInteractionLayer (bilinear_type='all') Trainium2 Bass kernel.

Contract: kernel(inputs=[2048,40,64] f32, w=[64,64] f32) -> [2048, 49920] f32,
matching

    xw  = einsum('bfd,de->bfe', inputs, w)
    p   = xw[:, I, :] * inputs[:, J, :]   # (I, J) = triu_indices(40, k=1)
    out = p.reshape(B, -1)

Data-parallel over 8 NeuronCores: batch 2048 -> 8 x 256, W replicated.

The kernel is HBM-write bound (51 MB of output per core). Writes are issued
as merged multi-block chunks (5-35 KB descriptors) to amortize per-descriptor
overhead. SDMA engine 15 (serving SBUF partitions 92-95/124-127 via the port
swizzle port_id = ((p>>2)&7)<<1 | ((p>>6)&1)) runs ~15% slower than engines
0-14 and paces the whole kernel under an even round-robin, so the tail
pair-blocks (i=25..38, 13.5% of each row) of the 8 port-15 rows per 128-row
tile are computed redundantly in stride-4 "tenant" partitions 0,4,...,28
(ports 0,2,...,14) and written from there, rebalancing bytes away from
engine 15. Tail multiplies run on the otherwise-idle GpSimd engine; main
blocks (i=0..24) stay on DVE.
"""

import numpy as np
from contextlib import ExitStack

import concourse.bass as bass  # noqa: F401  (registers engines)
import concourse.bacc as bacc
import concourse.tile as tile
import concourse.mybir as mybir
from concourse.bass_utils import run_bass_kernel_spmd

B = 2048
F = 40
D = 64
NCORES = 8
BS = B // NCORES                   # 256 rows per core
PAIRS = F * (F - 1) // 2           # 780
OUT_W = PAIRS * D                  # 49920
FD = F * D                         # 2560
DT = mybir.dt.float32

BLOCK_LEN = [F - 1 - i for i in range(F - 1)]
BLOCK_OFF = np.concatenate([[0], np.cumsum(BLOCK_LEN)[:-1]]).tolist()

# leading-field blocks grouped into chunk DMAs (contiguous output col ranges)
MAIN_GROUPS = [
    [0],
    [1, 2],
    [3, 4, 5],
    [6, 7, 8, 9],
    [10, 11, 12, 13, 14],
    [15, 16, 17, 18, 19],
    [20, 21, 22, 23, 24],
]
TAIL_GROUPS = [
    [25, 26, 27],
    [28, 29, 30, 31, 32],
    [33, 34, 35, 36, 37, 38],
]
TAIL_F0 = 24                       # x_ten holds fields 24..39

_CACHE = {}


def _grp_cols(g):
    c0 = BLOCK_OFF[g[0]] * D
    c1 = (BLOCK_OFF[g[-1]] + F - 1 - g[-1]) * D
    return c0, c1


def _build(bs: int):
    assert bs % 128 == 0
    ntiles = bs // 128
    nc = bacc.Bacc("TRN2", target_bir_lowering=False, debug=False)

    x_dram = nc.dram_tensor("x", [bs, F, D], DT, kind="ExternalInput").ap()
    w_dram = nc.dram_tensor("w", [D, D], DT, kind="ExternalInput").ap()
    id_dram = nc.dram_tensor("ident", [128, 128], DT, kind="ExternalInput").ap()
    out_dram = nc.dram_tensor("out", [bs, OUT_W], DT, kind="ExternalOutput").ap()

    x_flat = x_dram.rearrange("b f d -> b (f d)")

    with tile.TileContext(nc) as tc, ExitStack() as ctx:
        const_pool = ctx.enter_context(tc.tile_pool(name="const", bufs=1))
        x_pool = ctx.enter_context(tc.tile_pool(name="x", bufs=2))
        xten_pool = ctx.enter_context(tc.tile_pool(name="xten", bufs=2))
        xw_pool = ctx.enter_context(tc.tile_pool(name="xw", bufs=2))
        xwt_pool = ctx.enter_context(tc.tile_pool(name="xwt", bufs=2))
        tr_pool = ctx.enter_context(tc.tile_pool(name="tr", bufs=3))
        stage_pool = ctx.enter_context(tc.tile_pool(name="stage", bufs=3))
        psum_tr = ctx.enter_context(tc.tile_pool(name="psum_tr", bufs=2, space="PSUM"))
        psum_mm = ctx.enter_context(tc.tile_pool(name="psum_mm", bufs=4, space="PSUM"))

        ident = const_pool.tile([128, 128], DT)
        nc.scalar.dma_start(ident[:], id_dram)
        # W on both partition halves so the per-pair matmuls read lhsT and rhs
        # from the same base partition
        w_sb = const_pool.tile([128, D], DT)
        nc.scalar.dma_start(w_sb[0:D, :], w_dram)
        nc.scalar.dma_start(w_sb[D:128, :], w_dram)

        for t in range(ntiles):
            b0 = t * 128
            # ---- loads (scalar/ACT queue) ----
            x_t = x_pool.tile([128, FD], DT)
            # tail fields first: tail chunks + early transposes depend on them
            nc.scalar.dma_start(
                x_t[:, TAIL_F0 * D : FD], x_flat[b0 : b0 + 128, TAIL_F0 * D : FD]
            )
            nc.scalar.dma_start(
                x_t[:, 0 : TAIL_F0 * D], x_flat[b0 : b0 + 128, 0 : TAIL_F0 * D]
            )
            # tenant copy of x fields 24..39 for the 8 port-15 rows, placed on
            # stride-4 partitions 0,4,...,28 (ports 0,2,...,14)
            x_ten = xten_pool.tile([128, (F - TAIL_F0) * D], DT)
            nc.scalar.dma_start(
                x_ten[0:13:4, :], x_flat[b0 + 92 : b0 + 96, TAIL_F0 * D : FD]
            )
            nc.scalar.dma_start(
                x_ten[16:29:4, :], x_flat[b0 + 124 : b0 + 128, TAIL_F0 * D : FD]
            )

            xw_t = xw_pool.tile([128, (F - 1) * D], DT)       # fields 0..38
            xw_ten = xwt_pool.tile([128, 14 * D], DT)         # fields 25..38

            def nat_fp(fp):
                """transpose field-pair fp of x_t, matmul the covered fields."""
                tr_ps = psum_tr.tile([128, 128], DT)
                nc.tensor.transpose(
                    tr_ps[:], x_t[:, fp * 128 : (fp + 1) * 128], ident[:]
                )
                tr_sb = tr_pool.tile([128, 128], DT)
                nc.scalar.copy(tr_sb[:], tr_ps[:])
                for h in range(2):
                    f = 2 * fp + h
                    if f > F - 2:
                        continue
                    mm = psum_mm.tile([128, D], DT, tag="mm")
                    nc.tensor.matmul(
                        mm[:],
                        tr_sb[h * D : (h + 1) * D, :],
                        w_sb[h * D : (h + 1) * D, :],
                        start=True,
                        stop=True,
                    )
                    nc.scalar.copy(xw_t[:, f * D : (f + 1) * D], mm[:])

            def ten_fp(fp):
                """same for the tenant tile (fields 24+2fp, 25+2fp)."""
                tr_ps = psum_tr.tile([128, 128], DT)
                nc.tensor.transpose(
                    tr_ps[:], x_ten[:, fp * 128 : (fp + 1) * 128], ident[:]
                )
                tr_sb = tr_pool.tile([128, 128], DT)
                nc.scalar.copy(tr_sb[:], tr_ps[:])
                for h in range(2):
                    f = TAIL_F0 + 2 * fp + h
                    if f < 25 or f > F - 2:
                        continue
                    mm = psum_mm.tile([128, D], DT, tag="mm")
                    nc.tensor.matmul(
                        mm[:],
                        tr_sb[h * D : (h + 1) * D, :],
                        w_sb[h * D : (h + 1) * D, :],
                        start=True,
                        stop=True,
                    )
                    nc.scalar.copy(xw_ten[:, (f - 25) * D : (f - 24) * D], mm[:])

            # PE order: tail fields (for gpsimd tail chunks), first two main
            # field-pairs (for early DVE chunks), tenant pipeline, rest.
            for fp in range(12, 20):
                nat_fp(fp)
            for fp in (0, 1):
                nat_fp(fp)
            for fp in range(8):
                ten_fp(fp)
            for fp in range(2, 12):
                nat_fp(fp)

            # ---- gpsimd: natural tail chunks (written from non-port-15
            # partitions), then tenant tail chunks ----
            tail_nat = []
            for g in TAIL_GROUPS:
                c0, c1 = _grp_cols(g)
                st = stage_pool.tile([128, c1 - c0], DT)
                for i in g:
                    jn = F - 1 - i
                    lo = BLOCK_OFF[i] * D - c0
                    in0 = (
                        xw_t[:, i * D : (i + 1) * D]
                        .unsqueeze(1)
                        .broadcast_to([128, jn, D])
                    )
                    in1 = x_t[:, (i + 1) * D : FD].rearrange("p (j d) -> p j d", d=D)
                    nc.gpsimd.tensor_mul(
                        st[:, lo : lo + jn * D].rearrange("p (j d) -> p j d", d=D),
                        in0,
                        in1,
                    )
                tail_nat.append((st, c0, c1))
            tail_ten = []
            for g in TAIL_GROUPS:
                c0, c1 = _grp_cols(g)
                st = stage_pool.tile([128, c1 - c0], DT)
                for i in g:
                    jn = F - 1 - i
                    lo = BLOCK_OFF[i] * D - c0
                    in0 = (
                        xw_ten[:, (i - 25) * D : (i - 24) * D]
                        .unsqueeze(1)
                        .broadcast_to([128, jn, D])
                    )
                    in1 = x_ten[:, (i + 1 - TAIL_F0) * D :].rearrange(
                        "p (j d) -> p j d", d=D
                    )
                    nc.gpsimd.tensor_mul(
                        st[:, lo : lo + jn * D].rearrange("p (j d) -> p j d", d=D),
                        in0,
                        in1,
                    )
                tail_ten.append((st, c0, c1))

            # ---- DVE: main chunks ----
            main_st = []
            for g in MAIN_GROUPS:
                c0, c1 = _grp_cols(g)
                st = stage_pool.tile([128, c1 - c0], DT)
                for i in g:
                    jn = F - 1 - i
                    lo = BLOCK_OFF[i] * D - c0
                    in0 = (
                        xw_t[:, i * D : (i + 1) * D]
                        .unsqueeze(1)
                        .broadcast_to([128, jn, D])
                    )
                    in1 = x_t[:, (i + 1) * D : FD].rearrange("p (j d) -> p j d", d=D)
                    nc.vector.tensor_mul(
                        st[:, lo : lo + jn * D].rearrange("p (j d) -> p j d", d=D),
                        in0,
                        in1,
                    )
                main_st.append((st, c0, c1))

            # ---- output DMAs (sync queue), ordered by expected readiness ----
            def dma_tail_nat(k):
                st, c0, c1 = tail_nat[k]
                nc.sync.dma_start(out_dram[b0 + 0 : b0 + 92, c0:c1], st[0:92, :])
                nc.sync.dma_start(out_dram[b0 + 96 : b0 + 124, c0:c1], st[96:124, :])

            def dma_tail_ten(k):
                st, c0, c1 = tail_ten[k]
                nc.sync.dma_start(out_dram[b0 + 92 : b0 + 96, c0:c1], st[0:13:4, :])
                nc.sync.dma_start(
                    out_dram[b0 + 124 : b0 + 128, c0:c1], st[16:29:4, :]
                )

            def dma_main(k):
                st, c0, c1 = main_st[k]
                nc.sync.dma_start(out_dram[b0 : b0 + 128, c0:c1], st[:])

            dma_tail_nat(0)
            dma_main(0)
            dma_tail_nat(1)
            dma_tail_nat(2)
            dma_main(1)
            dma_tail_ten(0)
            dma_main(2)
            dma_tail_ten(1)
            dma_main(3)
            dma_tail_ten(2)
            dma_main(4)
            dma_main(5)
            dma_main(6)

    nc.compile()
    return nc


def _get_nc(bs: int):
    if bs not in _CACHE:
        _CACHE[bs] = _build(bs)
    return _CACHE[bs]


def _run(inputs: np.ndarray, w: np.ndarray, trace: bool = False):
    inputs = np.ascontiguousarray(inputs, dtype=np.float32)
    w = np.ascontiguousarray(w, dtype=np.float32)
    assert inputs.shape == (B, F, D) and w.shape == (D, D)
    nc = _get_nc(BS)
    ident = np.eye(128, dtype=np.float32)
    in_maps = [
        {"x": inputs[c * BS : (c + 1) * BS], "w": w, "ident": ident}
        for c in range(NCORES)
    ]
    res = run_bass_kernel_spmd(nc, in_maps, list(range(NCORES)), trace=trace)
    out = np.concatenate([res.results[c]["out"] for c in range(NCORES)], axis=0)
    return out, res


def kernel(inputs: np.ndarray, w: np.ndarray) -> np.ndarray:
    out, _ = _run(inputs, w)
    return out


# revision 3
# speedup vs baseline: 1.3333x; 1.3333x over previous
"""BiLinearInteractionLayer (bilinear_type='all') Trainium2 Bass kernel.

Contract: kernel(inputs=[2048,40,64] f32, w=[64,64] f32) -> [2048, 49920] f32,
matching

    xw  = einsum('bfd,de->bfe', inputs, w)
    p   = xw[:, I, :] * inputs[:, J, :]   # (I, J) = triu_indices(40, k=1)
    out = p.reshape(B, -1)

Data-parallel over 8 NeuronCores: batch 2048 -> 8 x 256, W replicated.

HBM-write bound (51 MB of output per core). Pair-blocks are merged into
chunk DMAs with 5-35 KB descriptors (vs 0.25-10 KB per-block) to amortize
per-descriptor overhead. SDMA engine 15 runs ~15% slower than engines 0-14
and paces an even 16-way round-robin, so the last pair-blocks (i=30..38,
2880 of 49920 cols) are written via partial-partition DMAs (92/28/4/4
rows), which the DGE observably routes to engines 0-13 only, shifting
~6% of the write bytes off engine 15. All multiplies stay on DVE (GpSimd
tensor ops grab the shared DVE/GpSimd SBUF port pair and block DVE).
"""

import numpy as np
from contextlib import ExitStack

import concourse.bass as bass  # noqa: F401  (registers engines)
import concourse.bacc as bacc
import concourse.tile as tile
import concourse.mybir as mybir
from concourse.bass_utils import run_bass_kernel_spmd

B = 2048
F = 40
D = 64
NCORES = 8
BS = B // NCORES                   # 256 rows per core
PAIRS = F * (F - 1) // 2           # 780
OUT_W = PAIRS * D                  # 49920
FD = F * D                         # 2560
DT = mybir.dt.float32

BLOCK_LEN = [F - 1 - i for i in range(F - 1)]
BLOCK_OFF = np.concatenate([[0], np.cumsum(BLOCK_LEN)[:-1]]).tolist()

# leading-field blocks grouped into chunk DMAs (contiguous output col ranges)
MAIN_GROUPS = [
    [0],
    [1, 2],
    [3, 4, 5],
    [6, 7, 8, 9],
    [10, 11, 12, 13, 14],
    [15, 16, 17, 18, 19],
    [20, 21, 22, 23, 24],
    [25, 26, 27],
    [28, 29],
]
TAIL_GROUP = list(range(30, 39))   # 2880 cols, written via partial DMAs
SPLIT_F = 30                       # x loads split: fields 30..39 first

_CACHE = {}


def _grp_cols(g):
    c0 = BLOCK_OFF[g[0]] * D
    c1 = (BLOCK_OFF[g[-1]] + F - 1 - g[-1]) * D
    return c0, c1


def _build(bs: int):
    assert bs % 128 == 0
    ntiles = bs // 128
    nc = bacc.Bacc("TRN2", target_bir_lowering=False, debug=False)

    x_dram = nc.dram_tensor("x", [bs, F, D], DT, kind="ExternalInput").ap()
    w_dram = nc.dram_tensor("w", [D, D], DT, kind="ExternalInput").ap()
    id_dram = nc.dram_tensor("ident", [128, 128], DT, kind="ExternalInput").ap()
    out_dram = nc.dram_tensor("out", [bs, OUT_W], DT, kind="ExternalOutput").ap()

    x_flat = x_dram.rearrange("b f d -> b (f d)")
    c_split = SPLIT_F * D

    with tile.TileContext(nc) as tc, ExitStack() as ctx:
        const_pool = ctx.enter_context(tc.tile_pool(name="const", bufs=1))
        x_pool = ctx.enter_context(tc.tile_pool(name="x", bufs=2))
        xw_pool = ctx.enter_context(tc.tile_pool(name="xw", bufs=2))
        tr_pool = ctx.enter_context(tc.tile_pool(name="tr", bufs=3))
        stage_pool = ctx.enter_context(tc.tile_pool(name="stage", bufs=3))
        psum_tr = ctx.enter_context(tc.tile_pool(name="psum_tr", bufs=2, space="PSUM"))
        psum_mm = ctx.enter_context(tc.tile_pool(name="psum_mm", bufs=4, space="PSUM"))

        ident = const_pool.tile([128, 128], DT)
        nc.scalar.dma_start(ident[:], id_dram)
        # W on both partition halves so the per-pair matmuls read lhsT and rhs
        # from the same base partition
        w_sb = const_pool.tile([128, D], DT)
        nc.scalar.dma_start(w_sb[0:D, :], w_dram)
        nc.scalar.dma_start(w_sb[D:128, :], w_dram)

        for t in range(ntiles):
            b0 = t * 128
            x_t = x_pool.tile([128, FD], DT)
            # tail fields first: the tail chunk and its partial DMAs depend
            # only on them, so the write stream starts early
            nc.scalar.dma_start(
                x_t[:, c_split:FD], x_flat[b0 : b0 + 128, c_split:FD]
            )
            nc.scalar.dma_start(
                x_t[:, 0:c_split], x_flat[b0 : b0 + 128, 0:c_split]
            )

            xw_t = xw_pool.tile([128, (F - 1) * D], DT)       # fields 0..38

            def nat_fp(fp):
                """transpose field-pair fp of x_t, matmul the covered fields."""
                tr_ps = psum_tr.tile([128, 128], DT)
                nc.tensor.transpose(
                    tr_ps[:], x_t[:, fp * 128 : (fp + 1) * 128], ident[:]
                )
                tr_sb = tr_pool.tile([128, 128], DT)
                nc.scalar.copy(tr_sb[:], tr_ps[:])
                for h in range(2):
                    f = 2 * fp + h
                    if f > F - 2:
                        continue
                    mm = psum_mm.tile([128, D], DT, tag="mm")
                    nc.tensor.matmul(
                        mm[:],
                        tr_sb[h * D : (h + 1) * D, :],
                        w_sb[h * D : (h + 1) * D, :],
                        start=True,
                        stop=True,
                    )
                    nc.scalar.copy(xw_t[:, f * D : (f + 1) * D], mm[:])

            # PE order: tail field-pairs first (fields 30..39), then 0..29
            for fp in range(15, 20):
                nat_fp(fp)
            for fp in range(15):
                nat_fp(fp)

            def mul_block(st, i, c0):
                jn = F - 1 - i
                lo = BLOCK_OFF[i] * D - c0
                in0 = (
                    xw_t[:, i * D : (i + 1) * D]
                    .unsqueeze(1)
                    .broadcast_to([128, jn, D])
                )
                in1 = x_t[:, (i + 1) * D : FD].rearrange("p (j d) -> p j d", d=D)
                nc.vector.tensor_mul(
                    st[:, lo : lo + jn * D].rearrange("p (j d) -> p j d", d=D),
                    in0,
                    in1,
                )

            # DVE order: tail blocks first, then main chunks ascending
            tc0, tc1 = _grp_cols(TAIL_GROUP)
            st_tail = stage_pool.tile([128, tc1 - tc0], DT)
            for i in TAIL_GROUP:
                mul_block(st_tail, i, tc0)
            main_st = []
            for g in MAIN_GROUPS:
                c0, c1 = _grp_cols(g)
                st = stage_pool.tile([128, c1 - c0], DT)
                for i in g:
                    mul_block(st, i, c0)
                main_st.append((st, c0, c1))

            # ---- output DMAs (sync queue) ----
            # tail via partial-partition DMAs (92/28/4/4 rows): the DGE routes
            # these to engines 0-13, sparing slow engine 15
            nc.sync.dma_start(out_dram[b0 + 0 : b0 + 92, tc0:tc1], st_tail[0:92, :])
            nc.sync.dma_start(
                out_dram[b0 + 96 : b0 + 124, tc0:tc1], st_tail[96:124, :]
            )
            nc.sync.dma_start(
                out_dram[b0 + 92 : b0 + 96, tc0:tc1], st_tail[92:96, :]
            )
            nc.sync.dma_start(
                out_dram[b0 + 124 : b0 + 128, tc0:tc1], st_tail[124:128, :]
            )
            for st, c0, c1 in main_st:
                nc.sync.dma_start(out_dram[b0 : b0 + 128, c0:c1], st[:])

    nc.compile()
    return nc


def _get_nc(bs: int):
    if bs not in _CACHE:
        _CACHE[bs] = _build(bs)
    return _CACHE[bs]


def _run(inputs: np.ndarray, w: np.ndarray, trace: bool = False):
    inputs = np.ascontiguousarray(inputs, dtype=np.float32)
    w = np.ascontiguousarray(w, dtype=np.float32)
    assert inputs.shape == (B, F, D) and w.shape == (D, D)
    nc = _get_nc(BS)
    ident = np.eye(128, dtype=np.float32)
    in_maps = [
        {"x": inputs[c * BS : (c + 1) * BS], "w": w, "ident": ident}
        for c in range(NCORES)
    ]
    res = run_bass_kernel_spmd(nc, in_maps, list(range(NCORES)), trace=trace)
    out = np.concatenate([res.results[c]["out"] for c in range(NCORES)], axis=0)
    return out, res


def kernel(inputs: np.ndarray, w: np.ndarray) -> np.ndarray:
    out, _ = _run(inputs, w)
    return out


# revision 4
# speedup vs baseline: 1.3894x; 1.0420x over previous
"""BiLinearInteractionLayer (bilinear_type='all') Trainium2 Bass kernel.

Contract: kernel(inputs=[2048,40,64] f32, w=[64,64] f32) -> [2048, 49920] f32,
matching

    xw  = einsum('bfd,de->bfe', inputs, w)
    p   = xw[:, I, :] * inputs[:, J, :]   # (I, J) = triu_indices(40, k=1)
    out = p.reshape(B, -1)

Data-parallel over 8 NeuronCores: batch 2048 -> 8 x 256, W replicated.

HBM-write bound (51 MB of output per core). Pair-blocks are merged into
chunk DMAs with 10-28 KB descriptors to amortize per-descriptor overhead.
SDMA engine 15 runs ~15% slower than engines 0-14 and paces an even 16-way
round-robin, so the last pair-blocks (i=28..38, 4224 of 49920 cols) are
written via sixteen-row partial-partition DMAs, which the DGE routes to
low-numbered engines, shifting tail bytes off engine 15. The partials are
interleaved between main-chunk DMAs to avoid convoys (one big
few-engine DMA blocking the FIFO queue). All multiplies stay on DVE
(GpSimd tensor ops grab the shared DVE/GpSimd SBUF port and block DVE).
"""

import numpy as np
from contextlib import ExitStack

import concourse.bass as bass  # noqa: F401  (registers engines)
import concourse.bacc as bacc
import concourse.tile as tile
import concourse.mybir as mybir
from concourse.bass_utils import run_bass_kernel_spmd

B = 2048
F = 40
D = 64
NCORES = 8
BS = B // NCORES                   # 256 rows per core
PAIRS = F * (F - 1) // 2           # 780
OUT_W = PAIRS * D                  # 49920
FD = F * D                         # 2560
DT = mybir.dt.float32

BLOCK_LEN = [F - 1 - i for i in range(F - 1)]
BLOCK_OFF = np.concatenate([[0], np.cumsum(BLOCK_LEN)[:-1]]).tolist()

# leading-field blocks grouped into chunk DMAs (contiguous output col ranges)
MAIN_GROUPS = [
    [0],
    [1, 2],
    [3, 4, 5],
    [6, 7],
    [8, 9],
    [10, 11, 12],
    [13, 14],
    [15, 16, 17, 18, 19],
    [20, 21, 22, 23, 24],
    [25, 26, 27],
]
TAIL_GROUP = list(range(28, 39))   # 4224 cols, written via 16-row partials
SPLIT_F = 28                       # x loads split: fields 28..39 first

_CACHE = {}


def _grp_cols(g):
    c0 = BLOCK_OFF[g[0]] * D
    c1 = (BLOCK_OFF[g[-1]] + F - 1 - g[-1]) * D
    return c0, c1


def _build(bs: int):
    assert bs % 128 == 0
    ntiles = bs // 128
    nc = bacc.Bacc("TRN2", target_bir_lowering=False, debug=False)

    x_dram = nc.dram_tensor("x", [bs, F, D], DT, kind="ExternalInput").ap()
    w_dram = nc.dram_tensor("w", [D, D], DT, kind="ExternalInput").ap()
    id_dram = nc.dram_tensor("ident", [128, 128], DT, kind="ExternalInput").ap()
    out_dram = nc.dram_tensor("out", [bs, OUT_W], DT, kind="ExternalOutput").ap()

    x_flat = x_dram.rearrange("b f d -> b (f d)")
    c_split = SPLIT_F * D

    with tile.TileContext(nc) as tc, ExitStack() as ctx:
        const_pool = ctx.enter_context(tc.tile_pool(name="const", bufs=1))
        x_pool = ctx.enter_context(tc.tile_pool(name="x", bufs=2))
        xw_pool = ctx.enter_context(tc.tile_pool(name="xw", bufs=2))
        tr_pool = ctx.enter_context(tc.tile_pool(name="tr", bufs=3))
        stage_pool = ctx.enter_context(tc.tile_pool(name="stage", bufs=4))
        tail_pool = ctx.enter_context(tc.tile_pool(name="tailst", bufs=2))
        psum_tr = ctx.enter_context(tc.tile_pool(name="psum_tr", bufs=2, space="PSUM"))
        psum_mm = ctx.enter_context(tc.tile_pool(name="psum_mm", bufs=4, space="PSUM"))

        ident = const_pool.tile([128, 128], DT)
        nc.scalar.dma_start(ident[:], id_dram)
        # W on both partition halves so the per-pair matmuls read lhsT and rhs
        # from the same base partition
        w_sb = const_pool.tile([128, D], DT)
        nc.scalar.dma_start(w_sb[0:D, :], w_dram)
        nc.scalar.dma_start(w_sb[D:128, :], w_dram)

        for t in range(ntiles):
            b0 = t * 128
            x_t = x_pool.tile([128, FD], DT)
            # tail fields first: the tail chunk and early transposes depend
            # only on them, so the write stream starts early
            nc.scalar.dma_start(
                x_t[:, c_split:FD], x_flat[b0 : b0 + 128, c_split:FD]
            )
            nc.scalar.dma_start(
                x_t[:, 0:c_split], x_flat[b0 : b0 + 128, 0:c_split]
            )

            xw_t = xw_pool.tile([128, (F - 1) * D], DT)       # fields 0..38

            def nat_fp(fp):
                """transpose field-pair fp of x_t, matmul the covered fields."""
                tr_ps = psum_tr.tile([128, 128], DT)
                nc.tensor.transpose(
                    tr_ps[:], x_t[:, fp * 128 : (fp + 1) * 128], ident[:]
                )
                tr_sb = tr_pool.tile([128, 128], DT)
                nc.scalar.copy(tr_sb[:], tr_ps[:])
                for h in range(2):
                    f = 2 * fp + h
                    if f > F - 2:
                        continue
                    mm = psum_mm.tile([128, D], DT, tag="mm")
                    nc.tensor.matmul(
                        mm[:],
                        tr_sb[h * D : (h + 1) * D, :],
                        w_sb[h * D : (h + 1) * D, :],
                        start=True,
                        stop=True,
                    )
                    nc.scalar.copy(xw_t[:, f * D : (f + 1) * D], mm[:])

            # PE order: tail field-pairs first (fields 28..39), then 0..27
            for fp in range(14, 20):
                nat_fp(fp)
            for fp in range(14):
                nat_fp(fp)

            def mul_block(st, i, c0):
                jn = F - 1 - i
                lo = BLOCK_OFF[i] * D - c0
                in0 = (
                    xw_t[:, i * D : (i + 1) * D]
                    .unsqueeze(1)
                    .broadcast_to([128, jn, D])
                )
                in1 = x_t[:, (i + 1) * D : FD].rearrange("p (j d) -> p j d", d=D)
                nc.vector.tensor_mul(
                    st[:, lo : lo + jn * D].rearrange("p (j d) -> p j d", d=D),
                    in0,
                    in1,
                )

            # DVE order: tail blocks first, then main chunks ascending
            tc0, tc1 = _grp_cols(TAIL_GROUP)
            st_tail = tail_pool.tile([128, tc1 - tc0], DT)
            for i in TAIL_GROUP:
                mul_block(st_tail, i, tc0)
            main_st = []
            for g in MAIN_GROUPS:
                c0, c1 = _grp_cols(g)
                st = stage_pool.tile([128, c1 - c0], DT)
                for i in g:
                    mul_block(st, i, c0)
                main_st.append((st, c0, c1))

            # ---- output DMAs (sync queue) ----
            # tail via 16-row partials routed to low engines (sparing slow
            # engine 15), interleaved with full-128 main chunks so no single
            # few-engine DMA convoys the FIFO queue
            def dma_tail(k):
                r0 = 16 * k
                nc.sync.dma_start(
                    out_dram[b0 + r0 : b0 + r0 + 16, tc0:tc1],
                    st_tail[r0 : r0 + 16, :],
                )

            def dma_main(k):
                st, c0, c1 = main_st[k]
                nc.sync.dma_start(out_dram[b0 : b0 + 128, c0:c1], st[:])

            dma_tail(0)
            dma_tail(1)
            for k in range(6):
                dma_main(k)
                dma_tail(k + 2)
            for k in range(6, 10):
                dma_main(k)

    nc.compile()
    return nc


def _get_nc(bs: int):
    if bs not in _CACHE:
        _CACHE[bs] = _build(bs)
    return _CACHE[bs]


def _run(inputs: np.ndarray, w: np.ndarray, trace: bool = False):
    inputs = np.ascontiguousarray(inputs, dtype=np.float32)
    w = np.ascontiguousarray(w, dtype=np.float32)
    assert inputs.shape == (B, F, D) and w.shape == (D, D)
    nc = _get_nc(BS)
    ident = np.eye(128, dtype=np.float32)
    in_maps = [
        {"x": inputs[c * BS : (c + 1) * BS], "w": w, "ident": ident}
        for c in range(NCORES)
    ]
    res = run_bass_kernel_spmd(nc, in_maps, list(range(NCORES)), trace=trace)
    out = np.concatenate([res.results[c]["out"] for c in range(NCORES)], axis=0)
    return out, res


def kernel(inputs: np.ndarray, w: np.ndarray) -> np.ndarray:
    out, _ = _run(inputs, w)
    return out


# revision 5
# speedup vs baseline: 1.4958x; 1.0766x over previous
"""BiLinearInteractionLayer (bilinear_type='all') Trainium2 Bass kernel.

Contract: kernel(inputs=[2048,40,64] f32, w=[64,64] f32) -> [2048, 49920] f32,
matching

    xw  = einsum('bfd,de->bfe', inputs, w)
    p   = xw[:, I, :] * inputs[:, J, :]   # (I, J) = triu_indices(40, k=1)
    out = p.reshape(B, -1)

Data-parallel over 8 NeuronCores: batch 2048 -> 8 x 256, W replicated.
Per core, each 128-row batch tile:
  - x tile [128, 2560] DMAs to SBUF (tail fields first so the small tail
    pair-blocks can start immediately)
  - PE transposes field pairs ([128,128] -> PSUM), ACT copies to SBUF,
    PE matmuls against replicated W (one PSUM tile per matmul), ACT
    copies xw to SBUF
  - per leading field i: one DVE broadcast-multiply of xw[:, i-block]
    against x[:, j>i], then one DMA of the [128, (39-i)*64] block
    straight to its contiguous slice of the output row
The kernel runs at the HBM-per-core write wall (~360 GB/s aggregate,
51 MB of output per core). SDMA engine 15 is ~15% slower than engines
0-14, so under the even 16-way round-robin it straggles ~7 us after the
other engines drain. To let it finish with the pack, the last pair-blocks
(i=34..38, 960 cols) are staged and written at the very end of the queue
via partial-partition DMAs (92/28/4/4 rows), which the DGE routes to
engines 0-13 only.
"""

import numpy as np
from contextlib import ExitStack

import concourse.bass as bass  # noqa: F401  (registers engines)
import concourse.bacc as bacc
import concourse.tile as tile
import concourse.mybir as mybir
from concourse.bass_utils import run_bass_kernel_spmd

B = 2048
F = 40
D = 64
NCORES = 8
BS = B // NCORES                   # 256 rows per core
PAIRS = F * (F - 1) // 2           # 780
OUT_W = PAIRS * D                  # 49920
FD = F * D                         # 2560
DT = mybir.dt.float32

BLOCK_LEN = [F - 1 - i for i in range(F - 1)]
BLOCK_OFF = np.concatenate([[0], np.cumsum(BLOCK_LEN)[:-1]]).tolist()

# tail field-pairs first: their pair-blocks are small and depend only on
# the tail x chunk, so the output DMA stream starts earliest
SPLIT_F = 30
FP_ORDER = list(range(SPLIT_F // 2, F // 2)) + list(range(SPLIT_F // 2))

TAIL_I0 = 34                       # blocks 34..38 staged, written at the end
TAIL_C0 = BLOCK_OFF[TAIL_I0] * D   # output col of the staged tail
TAIL_COLS = OUT_W - TAIL_C0        # 960

_CACHE = {}


def _build(bs: int):
    assert bs % 128 == 0
    ntiles = bs // 128
    nc = bacc.Bacc("TRN2", target_bir_lowering=False, debug=False)

    x_dram = nc.dram_tensor("x", [bs, F, D], DT, kind="ExternalInput").ap()
    w_dram = nc.dram_tensor("w", [D, D], DT, kind="ExternalInput").ap()
    id_dram = nc.dram_tensor("ident", [128, 128], DT, kind="ExternalInput").ap()
    out_dram = nc.dram_tensor("out", [bs, OUT_W], DT, kind="ExternalOutput").ap()

    x_flat = x_dram.rearrange("b f d -> b (f d)")
    c0 = SPLIT_F * D

    with tile.TileContext(nc) as tc, ExitStack() as ctx:
        const_pool = ctx.enter_context(tc.tile_pool(name="const", bufs=1))
        x_pool = ctx.enter_context(tc.tile_pool(name="x", bufs=2))
        xw_pool = ctx.enter_context(tc.tile_pool(name="xw", bufs=2))
        tr_pool = ctx.enter_context(tc.tile_pool(name="tr", bufs=3))
        stage_pool = ctx.enter_context(tc.tile_pool(name="stage", bufs=10))
        tail_pool = ctx.enter_context(tc.tile_pool(name="tailst", bufs=ntiles))
        psum_tr = ctx.enter_context(tc.tile_pool(name="psum_tr", bufs=2, space="PSUM"))
        psum_mm = ctx.enter_context(tc.tile_pool(name="psum_mm", bufs=4, space="PSUM"))

        ident = const_pool.tile([128, 128], DT)
        nc.scalar.dma_start(ident[:], id_dram)
        # W on both partition halves so the two per-pair matmuls read lhsT
        # and rhs from the same base partition
        w_sb = const_pool.tile([128, D], DT)
        nc.scalar.dma_start(w_sb[0:D, :], w_dram)
        nc.scalar.dma_start(w_sb[D:128, :], w_dram)

        x_tiles = []
        for t in range(ntiles):
            b0 = t * 128
            x_t = x_pool.tile([128, FD], DT)
            x_tiles.append(x_t)
            # tail fields first (sync ring), rest on the scalar ring
            nc.sync.dma_start(x_t[:, c0:FD], x_flat[b0 : b0 + 128, c0:FD])
            nc.scalar.dma_start(x_t[:, 0:c0], x_flat[b0 : b0 + 128, 0:c0])

        tails = []
        for t in range(ntiles):
            b0 = t * 128
            x_t = x_tiles[t]
            xw_t = xw_pool.tile([128, FD], DT)
            st_tail = tail_pool.tile([128, TAIL_COLS], DT)
            tails.append((b0, st_tail))
            for fp in FP_ORDER:
                tr_ps = psum_tr.tile([128, 128], DT)
                nc.tensor.transpose(
                    tr_ps[:], x_t[:, fp * 128 : (fp + 1) * 128], ident[:]
                )
                tr_sb = tr_pool.tile([128, 128], DT)
                nc.scalar.copy(tr_sb[:], tr_ps[:])
                for h in range(2):
                    i = 2 * fp + h
                    mm = psum_mm.tile([128, D], DT, tag="mm")
                    nc.tensor.matmul(
                        mm[:],
                        tr_sb[h * D : (h + 1) * D, :],
                        w_sb[h * D : (h + 1) * D, :],
                        start=True,
                        stop=True,
                    )
                    nc.scalar.copy(xw_t[:, i * D : (i + 1) * D], mm[:])
                for h in range(2):
                    i = 2 * fp + h
                    if i > F - 2:
                        continue  # field 39 never leads a pair
                    jn = F - 1 - i
                    in0 = (
                        xw_t[:, i * D : (i + 1) * D]
                        .unsqueeze(1)
                        .broadcast_to([128, jn, D])
                    )
                    in1 = x_t[:, (i + 1) * D : FD].rearrange(
                        "p (j d) -> p j d", d=D
                    )
                    if i >= TAIL_I0:
                        # staged: written at the very end via partials so
                        # slow engine 15 can finish with the pack
                        lo = BLOCK_OFF[i] * D - TAIL_C0
                        nc.vector.tensor_mul(
                            st_tail[:, lo : lo + jn * D].rearrange(
                                "p (j d) -> p j d", d=D
                            ),
                            in0,
                            in1,
                        )
                        continue
                    st = stage_pool.tile([128, jn * D], DT)
                    nc.vector.tensor_mul(
                        st[:].rearrange("p (j d) -> p j d", d=D), in0, in1
                    )
                    nc.sync.dma_start(
                        out_dram[
                            b0 : b0 + 128,
                            BLOCK_OFF[i] * D : (BLOCK_OFF[i] + jn) * D,
                        ],
                        st[:],
                    )

        # endgame: staged tails via partial-partition DMAs (engines 0-13)
        for b0, st_tail in tails:
            nc.sync.dma_start(
                out_dram[b0 + 0 : b0 + 92, TAIL_C0:OUT_W], st_tail[0:92, :]
            )
            nc.sync.dma_start(
                out_dram[b0 + 96 : b0 + 124, TAIL_C0:OUT_W], st_tail[96:124, :]
            )
            nc.sync.dma_start(
                out_dram[b0 + 92 : b0 + 96, TAIL_C0:OUT_W], st_tail[92:96, :]
            )
            nc.sync.dma_start(
                out_dram[b0 + 124 : b0 + 128, TAIL_C0:OUT_W], st_tail[124:128, :]
            )

    nc.compile()
    return nc


def _get_nc(bs: int):
    if bs not in _CACHE:
        _CACHE[bs] = _build(bs)
    return _CACHE[bs]


def _run(inputs: np.ndarray, w: np.ndarray, trace: bool = False):
    inputs = np.ascontiguousarray(inputs, dtype=np.float32)
    w = np.ascontiguousarray(w, dtype=np.float32)
    assert inputs.shape == (B, F, D) and w.shape == (D, D)
    nc = _get_nc(BS)
    ident = np.eye(128, dtype=np.float32)
    in_maps = [
        {"x": inputs[c * BS : (c + 1) * BS], "w": w, "ident": ident}
        for c in range(NCORES)
    ]
    res = run_bass_kernel_spmd(nc, in_maps, list(range(NCORES)), trace=trace)
    out = np.concatenate([res.results[c]["out"] for c in range(NCORES)], axis=0)
    return out, res


def kernel(inputs: np.ndarray, w: np.ndarray) -> np.ndarray:
    out, _ = _run(inputs, w)
    return out
